# revision 1
# baseline (speedup 1.0000x reference)
"""AdderNet 2D conv (L1-distance "convolution") on 8 TRN2 NeuronCores.

Reference computation:
    X_col = unfold(x, k=3, stride=1, pad=1)      # (N, D, P)  D=576, P=196
    out[n, f, p] = -sum_d |W_col[f, d] - X_col[n, d, p]|

Distribution: filter-parallel - core i computes filters f in [8i, 8i+8)
for the FULL batch (no collectives; host concatenates filter slices).
This makes the per-instruction free dim N*P = 3136, which amortizes
per-instruction overhead far better than batch-parallel (392).

Per-core algorithm (raw Bass; one inline sync-wait per instruction,
standalone wait_ge elsewhere):

  -sum_d |x-w|  =  -sum_d x  + sum_d w  + 2*sum_d min(x-w, 0)

  - Host im2col, d (patch dim, 576, (kh,kw,c)-ordered) on SBUF
    partitions: FOUR full 128-row chunks plus one FOLDED half-chunk:
    d 512:576 for positions 0:1568 on partitions 0:64 and for
    positions 1568:3136 on partitions 64:128 -> a (128, 1568) tile,
    halving that chunk's elementwise time.  All tiles dense bf16
    (window DMAs shatter into 28-byte descriptors).
  - DMA plan (measured: DMAs on ONE ring complete serially at ~2us
    fixed latency + transfer each, and a second ring's later DMAs
    starve behind a saturated first ring): the sync ring carries, in
    consumption order, ONE combined "xfoldng" tile (folded x chunk ++
    -sum_d X_col quarter rows ++ bias-init ones blocks ++ bf16 W
    columns) and then the four full x chunks.  Everything the
    pipeline start depends on lands in the FIRST serial DMA slot
    (~11us); no other ring is used for inputs.
  - W columns ship as bf16 and are converted on-chip to the fp32
    scalar1 operands by one tiny DVE op (tensor_scalar requires fp32
    scalars; ScalarE's activation bias reads the bf16 directly).
    Host computes sum_d W / sum_d x over the bf16-ROUNDED values so
    the identity terms match the device tiles.
  - PSUM is INITIALIZED to -sum_d x by a "bias-init" matmul whose
    stationary is a ones-block with 1.0 only at row 32c (selecting
    stream c's quarter row of the xfoldng tile), so the per-chunk
    -sum_d x matmul passes of the previous design (1/9 of all PE
    streaming) disappear, as does the zero-fill prologue.
  - Per (filter, chunk) unit, ONE elementwise instruction:
      VectorE: tensor_scalar(op0=sub W[f,.], op1=min 0)  -> min(x-w, 0)
        (4x_2p DVE perf mode: bf16, SBUF, unit stride; ~1.03us/full
        unit incl the inter-op SBUF read-write bubble)
      ScalarE: activation(Relu, scale=-1, bias=W[f,.])   -> relu(w-x)
        (1x rate, ~2.83us/full unit)
    Split by measured rates: ScalarE gets filters {3,7} in full
    chunks and {3,5,7} in the folded chunk (~24.6us) vs VectorE
    ~27.2us; GPSIMD tensor_scalar measured 45x slower than DVE and is
    NOT used for units (its queue runs all one-time memsets instead,
    so the DVE starts streaming the moment the xfoldng DMA lands).
  - TensorE reduces over partitions into PSUM, 4-way column-tiled:
    stream c (tile_position=(0, 32c)) computes ALL 8 filters for its
    private position quarter [784c, 784c+784), psum rows 32c..32c+8,
    banks {2c, 2c+1} (cols [1024c, 1024c+784)).  Streams never share
    a PSUM bank (concurrent accumulating matmuls on a shared bank
    corrupt it - measured).  Stationary blocks: full-chunk col j =
    +/-2; folded-chunk TOP (rows 0:64, streams 0-1) and BOT (rows
    64:128, streams 2-3) variants (signs follow the per-chunk engine
    split).  All matmuls keep the (128, 32) tile size (tiling-mode
    switches mid-stream corrupt results - measured).
  - HAM warmup: the PE clock sits gated at 1.2 GHz until ~3.4us of
    sustained activity; dummy matmuls into the (soon re-initialized)
    psum regions during the input-DMA wait open the clock gate before
    real accumulation begins (start=True re-clears the banks'
    has_written bits, so dummy garbage is harmless).
  - Evacuation per stream (pc-monotone matmul completion): psum rows
    + bias sum_d W[f] -> osb rows; streams 0-1 on ScalarE (activation
    Identity + bias) which then ships them itself (HWDGE), 2-3 on
    VectorE (tensor_scalar add) shipped by the sync queue.  One plain
    2-D DMA per stream to a stream-major DRAM output (a single 3-D
    gather AP gets mangled by the DMA AP optimizer); the host
    transposes stream-major -> filter-major.

kernel(x, W) accepts the FULL inputs and returns the FULL output.
"""

import numpy as np
import ml_dtypes

import concourse.bass as bass
from concourse import mybir
from concourse.bass_utils import run_bass_kernel_spmd

# Problem constants (hardcoded per harness rules)
N, C, H, W_SP = 16, 64, 14, 14
F = 64
KK = 3
PAD = 1
P = H * W_SP            # 196 output positions per image
POS = N * P             # 3136 total positions
D = C * KK * KK         # 576
N_CORES = 8
F_PER = F // N_CORES    # 8 filters per core
NFULL = 4               # full 128-row d-chunks (d 0:512)
FOLD_FD = POS // 2      # 1568: folded chunk free dim
NSTREAM = 4             # TensorE column-tiling streams
QPOS = POS // NSTREAM   # 784 positions per stream (its private quarter)
BANKC = 512             # psum bank capacity in f32
SUB = [(0, 512), (512, 272)]  # per-stream sub-slices (2 private banks)
RING_V = 8              # vector-produced tile ring
RING_S = 3              # scalar-produced tile ring
N_WARM = 7              # HAM-warmup dummy matmul rounds per stream

FP32 = mybir.dt.float32
BF16 = mybir.dt.bfloat16

WCOLS = (NFULL + 1) * F_PER  # 40 W columns (full: k*8+j; folded: 32+j)
# stat layout (bf16): [0:64) full blocks (8 per filter, col j = +/-2
# by that filter's engine in the full chunks), [64:128) folded TOP
# blocks (rows 0:64), [128:192) folded BOT blocks (rows 64:128),
# [192:200) always-zero block for the HAM-warmup dummies, [200:208)
# the +2 full-variant block for the DVE HALF of the split unit.
# The bias-init ones blocks ship inside the xfoldng DMA.
ST_FULL = 0
ST_TOP = 64
ST_BOT = 128
ST_ZERO = 192
ST_XTRA = 200
ST_N = 208
# engine assignment: ACT iff listed, GP iff listed; else DVE.  The
# SPLIT unit (filter 7, last chunk - an ACT filter elsewhere) is
# halved by position: DVE makes cols 0:1568 (consumed by streams 0-1,
# +2 stationary at ST_XTRA), ScalarE cols 1568:3136 (streams 2-3, the
# existing -2 full block), moving half an ACT unit to the 4x-rate
# DVE: DVE ~26.5us, ACT ~26.2us at measured rates (was 26.0/27.9).
ACT_BY_CHUNK = {0: (3, 5, 7), 1: (3, 7), 2: (3, 7), 3: (3, 7), 4: (3,)}
SPLIT_UNIT = (7, 4)
SPLIT_FD = POS // 2           # 1568: each half of the split unit
GP_BY_CHUNK = {k: () for k in range(5)}  # gpsimd tensor_scalar measured 45x slower than DVE - unusable
RING_G = 2
NEGX_OFF = FOLD_FD            # negx quarters inside the xfoldng tile
ONES_OFF = FOLD_FD + QPOS     # ones blocks inside the xfoldng tile
W_OFF = FOLD_FD + QPOS + 32   # bf16 W columns inside the xfoldng tile
XF_COLS = W_OFF + WCOLS + 1

# chunk ids: 0 = folded (FD 1568), 1..4 = full chunks 0..3 (FD 3136)
CHUNKS = [0, 1, 2, 3, 4]


def _chunk_fd(k):
    return FOLD_FD if k == 0 else POS


def build_bass():
    nc = bass.Bass()

    x_ext = nc.declare_dram_parameter("xcol", [NFULL, 128, POS], BF16,
                                      isOutput=False)
    # xfoldng (bf16, FIRST serial slot on the sync ring -- per-ring
    # DMAs complete serially at ~2us + transfer each, measured):
    #   cols 0:1568     folded x chunk
    #   cols 1568:2352  -sum_d x for position quarter c on row 32c
    #   cols 2352:2384  bias-init "ones" stationary blocks (col
    #                   2352+8c+j = 1.0 at row 32c only), so the
    #                   bias-init matmul writes psum[32c+j, p] =
    #                   -sum_d x[784c+p]
    #   cols 2384:2425  bf16 W columns (full 0:32 = k*8+j, folded
    #                   32:40, col 40 = sum_d W evacuation bias); DVE
    #                   converts them to fp32 on-chip (tensor_scalar's
    #                   scalar1 operand must be fp32), ScalarE reads
    #                   the bf16 directly as activation bias
    xf_ext = nc.declare_dram_parameter("xfoldng", [128, XF_COLS], BF16,
                                       isOutput=False)
    out_ext = nc.declare_dram_parameter("out", [NSTREAM, F_PER, QPOS],
                                        FP32, isOutput=True)

    # SBUF
    w_sb = nc.alloc_sbuf_tensor("w_sb", [128, WCOLS + 1], FP32)
    stat = nc.alloc_sbuf_tensor("stat", [128, ST_N], BF16)
    zmov = nc.alloc_sbuf_tensor("zmov", [128, BANKC], BF16)
    xfold = nc.alloc_sbuf_tensor("xfold_sb", [128, XF_COLS], BF16)
    xch = [nc.alloc_sbuf_tensor(f"xc{k}", [128, POS], BF16)
           for k in range(NFULL)]
    vring = [nc.alloc_sbuf_tensor(f"vb{r}", [128, POS], BF16)
             for r in range(RING_V)]
    sring = [nc.alloc_sbuf_tensor(f"sb{r}", [128, POS], BF16)
             for r in range(RING_S)]
    gring = [nc.alloc_sbuf_tensor(f"gb{r}", [128, POS], BF16)
             for r in range(RING_G)]
    osb = nc.alloc_sbuf_tensor("osb", [128, QPOS], FP32)

    # PSUM: 8 banks; stream c owns banks {2c, 2c+1} = cols
    # [1024c, 1024c+784) and computes ALL 8 filters (rows 32c..32c+8)
    # for its private position quarter [784c, 784c+784).
    psum = nc.alloc_psum_tensor("ps", [128, 8 * BANKC], FP32)

    def src_tile(k):
        return xfold if k == 0 else xch[k - 1]

    # unit order: per chunk, vector filters first, then the gpsimd
    # filter, then scalar filters, so the in-order PE consumer never
    # stalls early on a slow ACT/GP tile
    units = []
    for k in CHUNKS:
        vf = [j for j in range(F_PER)
              if j not in ACT_BY_CHUNK[k] and j not in GP_BY_CHUNK[k]
              and (j, k) != SPLIT_UNIT]
        units += ([(j, k) for j in vf] + [(j, k) for j in GP_BY_CHUNK[k]]
                  + [(j, k) for j in ACT_BY_CHUNK[k]])
    units.append(SPLIT_UNIT)  # both halves land last; minimal PE tail
    prod = {}   # (j,k) -> ("v"|"s"|"g", idx) or ("x", (v_idx, s_idx))
    nv = ns = ng = 0
    for (j, k) in units:
        if (j, k) == SPLIT_UNIT:
            prod[(j, k)] = ("x", (nv, ns))
            nv += 1
            ns += 1
        elif j in ACT_BY_CHUNK[k]:
            prod[(j, k)] = ("s", ns)
            ns += 1
        elif j in GP_BY_CHUNK[k]:
            prod[(j, k)] = ("g", ng)
            ng += 1
        else:
            prod[(j, k)] = ("v", nv)
            nv += 1

    def stat_block(j, k, c):
        """Stationary block for unit (j, chunk k) on stream c."""
        if k == 0:
            base = ST_TOP if c < 2 else ST_BOT
        else:
            base = ST_FULL
        return stat[:, base + 8 * j:base + 8 * j + 8]

    def mov_cols(k, c, so, sw):
        """Moving-operand column slice for stream c, sub (so, sw)."""
        if k == 0:
            off = QPOS * (c % 2) + so
        else:
            off = QPOS * c + so
        return off, off + sw

    with (
        nc.Block() as block,
        nc.semaphore("xf_sem") as xf_sem,
        nc.semaphore("x0_sem") as x0_sem,
        nc.semaphore("x1_sem") as x1_sem,
        nc.semaphore("x2_sem") as x2_sem,
        nc.semaphore("x3_sem") as x3_sem,
        nc.semaphore("out_sem") as out_sem,
        nc.semaphore("init_sem") as init_sem,  # zmov + stat zeroed
        nc.semaphore("dve_sem") as dve_sem,
        nc.semaphore("actp_sem") as actp_sem,
        nc.semaphore("pe_v_sem") as pe_v_sem,
        nc.semaphore("pe_s_sem") as pe_s_sem,
        nc.semaphore("evac2_sem") as evac2_sem,
        nc.semaphore("gp_sem") as gp_sem,
        nc.semaphore("pe_g_sem") as pe_g_sem,
        nc.semaphore("wconv_sem") as wconv_sem,
        nc.semaphore("st0_sem") as st0_sem,
        nc.semaphore("st1_sem") as st1_sem,
        nc.semaphore("st2_sem") as st2_sem,
        nc.semaphore("st3_sem") as st3_sem,
    ):
        xsem = {0: xf_sem, 1: x0_sem, 2: x1_sem, 3: x2_sem, 4: x3_sem}
        stsem = [st0_sem, st1_sem, st2_sem, st3_sem]

        @block.sync
        def _(sync: bass.BassEngine):
            # ALL input DMAs ride this one ring in consumption
            # order: a queue's descriptors drain in FIFO order, and
            # per-ring DMAs complete SERIALLY at ~2us + transfer each
            # (measured), so everything the pipeline start needs is
            # packed into the FIRST slot (xfoldng).
            sync.dma_start(out=xfold[:], in_=xf_ext[:],
                           single_packet=True).then_inc(xf_sem, 16)
            for k in range(NFULL):
                sync.dma_start(out=xch[k][:], in_=x_ext[k],
                               single_packet=True).then_inc(xsem[k + 1], 16)
            # output stores for streams 2-3 (evacuated by VectorE)
            for c in (2, 3):
                sync.wait_ge(evac2_sem, 1 + (c % 2))
                sync.dma_start(
                    out=out_ext[c],
                    in_=osb[32 * c:32 * c + F_PER, :],
                    single_packet=True,
                ).then_inc(out_sem, 16)
            sync.wait_ge(out_sem, 16 * NSTREAM)

        @block.vector
        def _(vector: bass.BassEngine):
            # all memsets live on the idle GpSimd queue; the DVE's only
            # pre-stream work is the W conversion, so its first unit
            # starts right as the xfoldng DMA lands.
            # Convert the bf16 W columns to the fp32 scalar1 operands
            # (same queue as the DVE units -> no semaphore needed for
            # them; gpsimd units wait on wconv_sem).
            vector.wait_ge(xf_sem, 16)
            vector.tensor_scalar(
                out=w_sb[:], in0=xfold[:, W_OFF:W_OFF + WCOLS + 1],
                scalar1=0.0, scalar2=None,
                op0=mybir.AluOpType.add).then_inc(wconv_sem, 1)
            seen = set()
            for (j, k) in units:
                kind, r = prod[(j, k)]
                if kind == "x":
                    r = r[0]        # v-half: cols 0:1568
                    c0, c1 = 0, SPLIT_FD
                elif kind == "v":
                    c0, c1 = 0, _chunk_fd(k)
                else:
                    continue
                if k not in seen:
                    seen.add(k)
                    vector.wait_ge(xsem[k], 16)
                if r >= RING_V:
                    vector.wait_ge(pe_v_sem, r - RING_V + 1)
                col = (32 + j) if k == 0 else ((k - 1) * F_PER + j)
                vector.tensor_scalar(
                    out=vring[r % RING_V][:, c0:c1],
                    in0=src_tile(k)[:, c0:c1],
                    scalar1=w_sb[:, col:col + 1], scalar2=0.0,
                    op0=mybir.AluOpType.subtract,
                    op1=mybir.AluOpType.min,
                ).then_inc(dve_sem, 1)
            # evacuate streams 2..3 (psum rows 32c..32c+8 + sum_d W)
            for c in range(2, NSTREAM):
                vector.wait_ge(stsem[c], 1)
                vector.tensor_scalar(
                    out=osb[32 * c:32 * c + F_PER, :],
                    in0=psum[32 * c:32 * c + F_PER,
                             1024 * c:1024 * c + QPOS],
                    scalar1=w_sb[32 * c:32 * c + F_PER, WCOLS:WCOLS + 1],
                    scalar2=None,
                    op0=mybir.AluOpType.add,
                ).then_inc(evac2_sem, 1)

        @block.scalar
        def _(scalar: bass.BassEngine):
            # touch the Relu table so the one-time ACT table load
            # overlaps the input DMAs instead of the first real unit
            scalar.activation(osb[0:1, 0:1], zmov[0:1, 0:1],
                              mybir.ActivationFunctionType.Relu,
                              bias=0.0, scale=1.0)
            scalar.wait_ge(xf_sem, 16)
            seen = set()
            for (j, k) in units:
                kind, r = prod[(j, k)]
                if kind == "x":
                    r = r[1]        # s-half: cols 1568:3136
                    c0, c1 = SPLIT_FD, POS
                elif kind == "s":
                    c0, c1 = 0, _chunk_fd(k)
                else:
                    continue
                if k not in seen:
                    seen.add(k)
                    scalar.wait_ge(xsem[k], 16)
                if r >= RING_S:
                    scalar.wait_ge(pe_s_sem, r - RING_S + 1)
                col = (32 + j) if k == 0 else ((k - 1) * F_PER + j)
                scalar.activation(
                    sring[r % RING_S][:, c0:c1], src_tile(k)[:, c0:c1],
                    mybir.ActivationFunctionType.Relu,
                    bias=xfold[:, W_OFF + col:W_OFF + col + 1], scale=-1.0,
                ).then_inc(actp_sem, 1)
            # evacuate streams 0..1 and ship them directly (HWDGE)
            for c in range(0, 2):
                scalar.wait_ge(stsem[c], 1)
                scalar.activation(
                    osb[32 * c:32 * c + F_PER, :],
                    psum[32 * c:32 * c + F_PER, 1024 * c:1024 * c + QPOS],
                    mybir.ActivationFunctionType.Identity,
                    bias=xfold[32 * c:32 * c + F_PER,
                               W_OFF + WCOLS:W_OFF + WCOLS + 1],
                    scale=1.0,
                )
                scalar.dma_start(
                    out=out_ext[c],
                    in_=osb[32 * c:32 * c + F_PER, :],
                    single_packet=True,
                ).then_inc(out_sem, 16)

        @block.gpsimd
        def _(gp: bass.BassEngine):
            # all one-time memsets live here (the queue is otherwise
            # idle), freeing the DVE to start streaming immediately
            gp.memset(zmov[:], 0.0)
            gp.memset(stat[:], 0.0).then_inc(init_sem, 1)
            # stationary filter blocks (sign = +2 for min-producing
            # DVE/GP tiles, -2 for relu-producing ScalarE tiles)
            for j in range(F_PER):
                fval = -2.0 if j in ACT_BY_CHUNK[1] else 2.0
                kval = -2.0 if j in ACT_BY_CHUNK[0] else 2.0
                gp.memset(stat[:, ST_FULL + 8 * j + j:
                               ST_FULL + 8 * j + j + 1], fval)
                gp.memset(stat[0:64, ST_TOP + 8 * j + j:
                               ST_TOP + 8 * j + j + 1], kval)
                gp.memset(stat[64:128, ST_BOT + 8 * j + j:
                               ST_BOT + 8 * j + j + 1], kval)
            # +2 full-variant block for the split unit's DVE half
            gp.memset(stat[:, ST_XTRA + SPLIT_UNIT[0]:
                           ST_XTRA + SPLIT_UNIT[0] + 1], 2.0)
            # third elementwise producer: min(x-w, 0) tiles, same as
            # DVE (stationary +2)
            gp.wait_ge(wconv_sem, 1)
            seen = set()
            for (j, k) in units:
                kind, r = prod[(j, k)]
                if kind != "g":
                    continue
                if k not in seen:
                    seen.add(k)
                    gp.wait_ge(xsem[k], 16)
                if r >= RING_G:
                    gp.wait_ge(pe_g_sem, r - RING_G + 1)
                fd = _chunk_fd(k)
                col = (32 + j) if k == 0 else ((k - 1) * F_PER + j)
                gp.tensor_scalar(
                    out=gring[r % RING_G][:, 0:fd],
                    in0=src_tile(k)[:, 0:fd],
                    scalar1=w_sb[:, col:col + 1], scalar2=0.0,
                    op0=mybir.AluOpType.subtract,
                    op1=mybir.AluOpType.min,
                ).then_inc(gp_sem, 1)

        @block.tensor
        def _(tensor: bass.BassEngine):
            tensor.wait_ge(init_sem, 1)  # zmov + ones ready
            # HAM warmup: keep the PE array busy through the input-DMA
            # wait so its clock gate opens (1.2 -> 2.4 GHz) before real
            # work.  Dummy targets are re-initialized below (start=True
            # clears the whole bank's has_written bits), so garbage is
            # fine.
            for _w in range(N_WARM):
                for c in range(NSTREAM):
                    tensor.matmul(
                        psum[32 * c:32 * c + F_PER,
                             1024 * c:1024 * c + BANKC],
                        stat[:, ST_ZERO:ST_ZERO + 8],
                        zmov[:, 0:BANKC],
                        start=True, stop=True, skip_group_check=True,
                        tile_position=(0, 32 * c),
                    )
            # bias-init: psum[32c+j, p] = -sum_d x[784c + p].  The
            # stationary ones-block (shipped inside xfoldng) has 1.0
            # only at row 32c, selecting that stream's quarter row.
            # Replaces both the zero-prologue and the per-chunk
            # -sum_d x matmul passes.
            tensor.wait_ge(xf_sem, 16)   # xfoldng DMA complete
            for c in range(NSTREAM):
                for (so, sw) in SUB:
                    tensor.matmul(
                        psum[32 * c:32 * c + F_PER,
                             1024 * c + so:1024 * c + so + sw],
                        xfold[:, ONES_OFF + 8 * c:ONES_OFF + 8 * c + 8],
                        xfold[:, NEGX_OFF + so:NEGX_OFF + so + sw],
                        start=True, stop=False, skip_group_check=True,
                        tile_position=(0, 32 * c),
                    )
            for (j, k) in units:
                kind, r = prod[(j, k)]
                if kind == "x":
                    # split unit: streams 0-1 read the DVE half (cols
                    # 0:1568, +2 stationary), streams 2-3 the ScalarE
                    # half (cols 1568:3136 of its own slot, -2); the
                    # waits happen per-half below
                    a = None
                elif kind == "v":
                    tensor.wait_ge(dve_sem, r + 1)
                    a = vring[r % RING_V]
                elif kind == "g":
                    tensor.wait_ge(gp_sem, r + 1)
                    a = gring[r % RING_G]
                else:
                    tensor.wait_ge(actp_sem, r + 1)
                    a = sring[r % RING_S]
                is_last = (j, k) == units[-1]
                for c in range(NSTREAM):
                    if kind == "x":
                        if c == 0:
                            tensor.wait_ge(dve_sem, r[0] + 1)
                            a = vring[r[0] % RING_V]
                        elif c == 2:
                            tensor.wait_ge(actp_sem, r[1] + 1)
                            a = sring[r[1] % RING_S]
                        blk = (stat[:, ST_XTRA:ST_XTRA + 8] if c < 2
                               else stat_block(j, k, c))
                    else:
                        blk = stat_block(j, k, c)
                    for si, (so, sw) in enumerate(SUB):
                        fin = c == NSTREAM - 1 and si == len(SUB) - 1
                        m0, m1 = mov_cols(k, c, so, sw)
                        mm = tensor.matmul(
                            psum[32 * c:32 * c + F_PER,
                                 1024 * c + so:1024 * c + so + sw],
                            blk,
                            a[:, m0:m1],
                            start=False, stop=is_last and fin,
                            skip_group_check=True,
                            tile_position=(0, 32 * c),
                        )
                        if is_last and si == len(SUB) - 1:
                            # stream c fully accumulated (pc order)
                            mm.then_inc(stsem[c], 1)
                        elif fin:
                            mm.then_inc(
                                pe_v_sem if kind == "v" else
                                pe_g_sem if kind == "g" else pe_s_sem, 1)

    return nc


def _prep_inputs(x: np.ndarray, W: np.ndarray):
    x = np.asarray(x, dtype=np.float32)
    W = np.asarray(W, dtype=np.float32)
    # Host im2col in (kh, kw, c) d-order
    xp = np.zeros((C, N, H + 2, W_SP + 2), np.float32)
    xp[:, :, PAD:PAD + H, PAD:PAD + W_SP] = x.transpose(1, 0, 2, 3)
    xc = np.zeros((D, POS), np.float32)
    for b in range(KK * KK):
        kh, kw = divmod(b, KK)
        xc[64 * b:64 * (b + 1), :] = (
            xp[:, :, kh:kh + H, kw:kw + W_SP].reshape(C, POS))
    xfull = (xc[:512].reshape(NFULL, 128, POS)).astype(ml_dtypes.bfloat16)
    # sums computed over the bf16-ROUNDED values the device actually
    # uses, so the -sum x / +sum w identity terms match the tiles
    nx = -np.asarray(xfull, np.float32).sum(axis=(0, 1)) \
        - np.asarray(xc[512:].astype(ml_dtypes.bfloat16),
                     np.float32).sum(axis=0)
    # W_col in (kh, kw, c) d-order, bf16-rounded: (F, 576)
    Wp = W.transpose(0, 2, 3, 1).reshape(F, KK * KK * C)
    Wpb = np.asarray(Wp.astype(ml_dtypes.bfloat16), np.float32)
    xfolds = []
    for i in range(N_CORES):
        xfold = np.zeros((128, XF_COLS), np.float32)
        xfold[0:64, 0:FOLD_FD] = xc[512:, 0:FOLD_FD]
        xfold[64:128, 0:FOLD_FD] = xc[512:, FOLD_FD:POS]
        for c in range(NSTREAM):
            xfold[32 * c, NEGX_OFF:NEGX_OFF + QPOS] = \
                nx[QPOS * c:QPOS * (c + 1)]
            for j in range(F_PER):
                xfold[32 * c, ONES_OFF + 8 * c + j] = 1.0
        for k in range(NFULL):
            blk = Wpb[F_PER * i:F_PER * (i + 1), 128 * k:128 * (k + 1)].T
            xfold[:, W_OFF + k * F_PER:W_OFF + (k + 1) * F_PER] = blk
        fb = Wpb[F_PER * i:F_PER * (i + 1), 512:D].T  # (64, 8)
        xfold[0:64, W_OFF + 32:W_OFF + 40] = fb
        xfold[64:128, W_OFF + 32:W_OFF + 40] = fb
        sw = Wpb[F_PER * i:F_PER * (i + 1), :].sum(axis=1)
        for c in range(NSTREAM):
            xfold[32 * c:32 * c + F_PER, W_OFF + WCOLS] = sw
        xfolds.append(xfold.astype(ml_dtypes.bfloat16))
    return xfull, xfolds


_CACHED_NC = None
LAST_RESULT = None  # BassKernelResults of the most recent run (for test.py)


def kernel(x: np.ndarray, W: np.ndarray, _trace: bool = False) -> np.ndarray:
    global _CACHED_NC, LAST_RESULT
    xfull, xfolds = _prep_inputs(x, W)
    if _CACHED_NC is None:
        _CACHED_NC = build_bass()
    nc = _CACHED_NC
    in_maps = [{"xcol": xfull, "xfoldng": xfolds[i]}
               for i in range(N_CORES)]
    res = run_bass_kernel_spmd(nc, in_maps, core_ids=list(range(N_CORES)),
                               trace=_trace)
    LAST_RESULT = res
    outs = [np.asarray(res.results[i]["out"], dtype=np.float32)
            .transpose(1, 0, 2).reshape(F_PER, POS)
            for i in range(N_CORES)]
    o = np.concatenate(outs, axis=0)                    # (64, 3136)
    o = (o.reshape(F, N, P).transpose(1, 0, 2)
          .reshape(N, F, H, W_SP).astype(np.float32))
    return o

